# revision 7
# baseline (speedup 1.0000x reference)
"""Causal multi-head attention (B=2, H=16, S=2048, D=128) on 8 TRN2 NeuronCores.

Sharding: batch*heads (32) split across 8 cores, 4 heads per core.
Per-head algorithm (fp16 matmuls / f32 accumulation), v2:
  - chunked dtype-cast DMA loads (f32 DRAM -> fp16 SBUF), prefetched a head ahead
  - PE-transpose Q,K 128x128 tiles to [d, s] layout (contraction on partitions)
  - scores computed transposed: S^T[k, q] so the PV matmul needs no P transpose
  - full k-chunks in 3-tile PSUM groups [128,1536]; the 4 diagonal k-chunks of
    each 512-q block as 4 row-matmuls packed tight [128,1280] (valid cols only)
  - P^T = exp(S^T/sqrt(D)) on ScalarE straight from PSUM -> SBUF fp16; no
    max-subtraction needed (scores ~N(0,1); the reference's -10000 mask
    underflows to exact 0 in exp, so hard zeros match it)
  - causal diagonal tiles masked by a 0/1 triangle multiply (GpSimd) after exp
  - O = sum_k P^T.T @ V_aug with a ones column appended to V -> last column of
    the accumulator is the softmax denominator; DVE reciprocal + tensor_scalar
  - per-q-block f32 stores
"""

import math

import numpy as np

import concourse.bass as bass
import concourse.tile as tile
from concourse import bacc, mybir
from concourse.bass_utils import run_bass_kernel_spmd

B, H, S, D = 2, 16, 2048, 128
N_CORES = 8
HPC = (B * H) // N_CORES  # heads per core
P = 128                   # partitions / head_dim / k-chunk
NT = S // P               # 16 k-chunks (s-tiles) per head
QB = 512                  # q-block width
NQB = S // QB             # 4 q-blocks per head
G = 3                     # full-group tiles per exp (3 PSUM banks)

FP16 = mybir.dt.float16
F32 = mybir.dt.float32
EXPFN = mybir.ActivationFunctionType.Exp
SCALE = 1.0 / math.sqrt(D)

_cache = {}


def _build_program():
    """Build (once) the single-core Bass/Tile program used SPMD on all cores."""
    if "nc" in _cache:
        return _cache["nc"]

    nc = bacc.Bacc("TRN2", target_bir_lowering=False, debug=False)

    q_d = nc.dram_tensor("q", [HPC * S, D], F32, kind="ExternalInput").ap()
    k_d = nc.dram_tensor("k", [HPC * S, D], F32, kind="ExternalInput").ap()
    v_d = nc.dram_tensor("v", [HPC * S, D], F32, kind="ExternalInput").ap()
    ident_d = nc.dram_tensor("ident", [P, P], FP16, kind="ExternalInput").ap()
    tri_d = nc.dram_tensor("tri", [P, P], FP16, kind="ExternalInput").ap()
    o_d = nc.dram_tensor("o", [HPC * S, D], F32, kind="ExternalOutput").ap()

    with tile.TileContext(nc) as tc:
        with (
            tc.tile_pool(name="consts", bufs=1) as consts,
            tc.tile_pool(name="qn", bufs=2) as qn_pool,
            tc.tile_pool(name="kn", bufs=2) as kn_pool,
            tc.tile_pool(name="qt", bufs=2) as qt_pool,
            tc.tile_pool(name="kt", bufs=2) as kt_pool,
            tc.tile_pool(name="vt", bufs=2) as vt_pool,
            tc.tile_pool(name="ptf", bufs=2) as ptf_pool,
            tc.tile_pool(name="ptd", bufs=2) as ptd_pool,
            tc.tile_pool(name="ostage", bufs=2) as ostage_pool,
            tc.tile_pool(name="rec", bufs=4) as rec_pool,
            tc.tile_pool(name="stp", bufs=2, space="PSUM") as st_pool,
            tc.tile_pool(name="ops", bufs=2, space="PSUM") as o_pool,
        ):
            ident = consts.tile([P, P], FP16)
            nc.sync.dma_start(ident[:], ident_d[:])
            tri = consts.tile([P, P], FP16)
            nc.sync.dma_start(tri[:], tri_d[:])

            heads = [dict() for _ in range(HPC)]

            def emit_load(h):
                t = heads[h]
                rows = slice(h * S, (h + 1) * S)
                q_h = q_d[rows, :].rearrange("(n p) d -> p n d", p=P)
                k_h = k_d[rows, :].rearrange("(n p) d -> p n d", p=P)
                v_h = v_d[rows, :].rearrange("(n p) d -> p n d", p=P)
                qn = qn_pool.tile([P, NT, P], FP16, name=f"qn{h}", tag="qn")
                kn = kn_pool.tile([P, NT, P], FP16, name=f"kn{h}", tag="kn")
                vt = vt_pool.tile([P, NT, P + 1], FP16, name=f"vt{h}", tag="vt")
                for c in range(4):
                    cs = slice(4 * c, 4 * c + 4)
                    nc.gpsimd.dma_start(kn[:, cs, :], k_h[:, cs, :])
                    nc.gpsimd.dma_start(qn[:, cs, :], q_h[:, cs, :])
                for c in range(2):
                    cs = slice(8 * c, 8 * c + 8)
                    nc.gpsimd.dma_start(vt[:, cs, 0:P], v_h[:, cs, :])
                nc.vector.memset(vt[:, :, P : P + 1], 1.0)
                t["qn"], t["kn"], t["vt"] = qn, kn, vt

            def emit_trans(h):
                t = heads[h]
                qt = qt_pool.tile([P, NT, P], FP16, name=f"qt{h}", tag="qt")
                kt = kt_pool.tile([P, NT, P], FP16, name=f"kt{h}", tag="kt")
                for c in range(4):
                    ts = st_pool.tile(
                        [P, 2, 2 * QB], FP16, name=f"ts{h}_{c}", tag="stp"
                    )
                    for i in range(4):
                        nc.tensor.transpose(
                            ts[:, 0, i * P : (i + 1) * P],
                            t["qn"][:, 4 * c + i, :],
                            ident[:],
                        )
                    for i in range(4):
                        nc.tensor.transpose(
                            ts[:, 1, i * P : (i + 1) * P],
                            t["kn"][:, 4 * c + i, :],
                            ident[:],
                        )
                    cs = slice(4 * c, 4 * c + 4)
                    nc.vector.tensor_copy(qt[:, cs, :], ts[:, 0, 0:QB])
                    nc.vector.tensor_copy(kt[:, cs, :], ts[:, 1, 0:QB])
                t["qt"], t["kt"] = qt, kt

            def emit_block(h, b):
                t = heads[h]
                qt, kt, vt = t["qt"], t["kt"], t["vt"]
                qt_b = qt[:, 4 * b : 4 * b + 4, :]  # [128, 512] moving operand
                nfull = 4 * b
                # --- full (entirely valid) k-chunks in G-tile psum groups ---
                ptf = None
                if nfull:
                    ptf = ptf_pool.tile(
                        [P, 12, QB], FP16, name=f"ptf{h}_{b}", tag="ptf"
                    )
                    for gs in range(0, nfull, G):
                        gw = min(G, nfull - gs)
                        stp = st_pool.tile(
                            [P, G, QB], F32, name=f"st{h}_{b}_{gs}", tag="stp"
                        )
                        for jj in range(gw):
                            nc.tensor.matmul(
                                stp[:, jj, :],
                                lhsT=kt[:, gs + jj, :],
                                rhs=qt_b,
                                start=True,
                                stop=True,
                            )
                        nc.scalar.activation(
                            ptf[:, gs : gs + gw, :],
                            stp[:, 0:gw, :],
                            EXPFN,
                            scale=SCALE,
                        )
                # --- diagonal k-chunks: one row-matmul per r, packed tight ---
                dstp = st_pool.tile([P, 1280], F32, name=f"dst{h}_{b}", tag="stp")
                ptd = ptd_pool.tile([P, 1280], FP16, name=f"ptd{h}_{b}", tag="ptd")
                # row order (0,3,1,2) packs to exactly 1280 cols without any
                # matmul crossing a 512-f32 psum bank boundary
                dcol = {}
                col = 0
                for r in (0, 3, 1, 2):
                    w = QB - P * r
                    nc.tensor.matmul(
                        dstp[:, col : col + w],
                        lhsT=kt[:, 4 * b + r, :],
                        rhs=qt_b[:, r:4, :],
                        start=True,
                        stop=True,
                    )
                    dcol[r] = col
                    col += w
                nc.scalar.activation(ptd[:], dstp[:], EXPFN, scale=SCALE)
                for r in range(4):
                    dslc = ptd[:, dcol[r] : dcol[r] + P]
                    nc.gpsimd.tensor_mul(dslc, dslc, tri[:])
                # --- O accumulation: full parts first, then diagonal parts ---
                otiles = [
                    o_pool.tile([P, 2, P + 1], F32, name=f"o{h}_{b}_{i}", tag="ops")
                    for i in range(2)
                ]
                for s in range(4):
                    ot = otiles[s // 2][:, s % 2, :]
                    for j in range(nfull):
                        nc.tensor.matmul(
                            ot,
                            lhsT=ptf[:, j, s * P : (s + 1) * P],
                            rhs=vt[:, j, :],
                            start=(j == 0),
                            stop=False,
                        )
                    for r in range(s + 1):
                        c0 = dcol[r] + (s - r) * P
                        nc.tensor.matmul(
                            ot,
                            lhsT=ptd[:, c0 : c0 + P],
                            rhs=vt[:, 4 * b + r, :],
                            start=(nfull == 0 and r == 0),
                            stop=(r == s),
                        )
                rows = slice(h * S, (h + 1) * S)
                o_h = o_d[rows, :].rearrange("(n p) d -> p n d", p=P)
                for s in range(4):
                    g = 4 * b + s
                    ot = otiles[s // 2][:, s % 2, :]
                    rec = rec_pool.tile([P, 1], F32, name=f"r{h}_{g}", tag="rec")
                    nc.vector.reciprocal(rec[:], ot[:, P : P + 1])
                    nc.vector.tensor_scalar_mul(
                        t["ostage"][:, g, :], ot[:, 0:P], rec[:]
                    )
                bs = slice(4 * b, 4 * b + 4)
                nc.sync.dma_start(o_h[:, bs, :], t["ostage"][:, bs, :])

            emit_load(0)
            emit_trans(0)
            for h in range(HPC):
                heads[h]["ostage"] = ostage_pool.tile(
                    [P, NT, P], F32, name=f"os{h}", tag="ostage"
                )
                for b in range(NQB):
                    emit_block(h, b)
                    if b == 0 and h + 1 < HPC:
                        emit_load(h + 1)
                    if b == 2 and h + 1 < HPC:
                        emit_trans(h + 1)

    nc.compile()
    _cache["nc"] = nc
    return nc


def _make_const_inputs():
    ident = np.eye(P, dtype=np.float16)
    # tri[kk, qq] = 1 where qq >= kk (valid causal positions in S^T layout)
    tri = np.triu(np.ones((P, P), dtype=np.float16))
    return ident, tri


def run_sharded(q, k, v, trace=False, **kw):
    """q,k,v: [B,H,S,D] f32 -> (out [B,H,S,D] f32, BassKernelResults)."""
    nc = _build_program()
    qf = np.ascontiguousarray(np.asarray(q, dtype=np.float32).reshape(B * H, S, D))
    kf = np.ascontiguousarray(np.asarray(k, dtype=np.float32).reshape(B * H, S, D))
    vf = np.ascontiguousarray(np.asarray(v, dtype=np.float32).reshape(B * H, S, D))
    ident, tri = _make_const_inputs()
    in_maps = []
    for c in range(N_CORES):
        hs = slice(c * HPC, (c + 1) * HPC)
        in_maps.append(
            {
                "q": qf[hs].reshape(HPC * S, D),
                "k": kf[hs].reshape(HPC * S, D),
                "v": vf[hs].reshape(HPC * S, D),
                "ident": ident,
                "tri": tri,
            }
        )
    res = run_bass_kernel_spmd(nc, in_maps, list(range(N_CORES)), trace=trace, **kw)
    outs = [res.results[c]["o"].reshape(HPC, S, D) for c in range(N_CORES)]
    full = np.concatenate(outs, axis=0).reshape(B, H, S, D)
    return full, res


def kernel(query_states, key_states, value_states):
    out, _ = run_sharded(query_states, key_states, value_states)
    return out.astype(np.float32)


# revision 8
# speedup vs baseline: 1.0464x; 1.0464x over previous
"""Causal multi-head attention (B=2, H=16, S=2048, D=128) on 8 TRN2 NeuronCores.

Sharding: batch*heads (32) split across 8 cores, 4 heads per core.
Per-head algorithm (fp16 matmuls / f32 accumulation), v3:
  - chunked HWDGE f32 loads for Q,K (descriptor gen off the GpSimd Q7);
    V loaded via one SWDGE cast-DMA (f32 -> fp16) with a ones column appended
  - PE-transpose Q,K 128x128 f32 tiles to [d, s] layout; the PSUM->SBUF copy
    casts to fp16 (DVE)
  - scores computed transposed: S^T[k, q] so the PV matmul needs no P transpose
  - full k-chunks in 3-tile PSUM groups [128,1536]; the 4 diagonal k-chunks of
    each 512-q block as 4 row-matmuls packed tight [128,1280] (valid cols only)
  - P^T = exp(S^T/sqrt(D)) on ScalarE straight from PSUM -> SBUF fp16; no
    max-subtraction needed (scores ~N(0,1); the reference's -10000 mask
    underflows to exact 0 in exp, so hard zeros match it)
  - causal diagonal tiles masked by a 0/1 triangle multiply (DVE) after exp
  - O = sum_k P^T.T @ V_aug with a ones column appended to V -> last column of
    the accumulator is the softmax denominator; DVE reciprocal + tensor_scalar
  - per-q-block f32 stores; O-phase of each block emitted one scores-unit late
    so PE keeps ScalarE fed across block/head boundaries
"""

import math

import numpy as np

import concourse.bass as bass
import concourse.tile as tile
from concourse import bacc, mybir
from concourse.bass_utils import run_bass_kernel_spmd

B, H, S, D = 2, 16, 2048, 128
N_CORES = 8
HPC = (B * H) // N_CORES  # heads per core
P = 128                   # partitions / head_dim / k-chunk
NT = S // P               # 16 k-chunks (s-tiles) per head
QB = 512                  # q-block width
NQB = S // QB             # 4 q-blocks per head
G = 3                     # full-group tiles per exp (3 PSUM banks)

FP16 = mybir.dt.float16
F32 = mybir.dt.float32
EXPFN = mybir.ActivationFunctionType.Exp
SCALE = 1.0 / math.sqrt(D)

_cache = {}


def _build_program():
    """Build (once) the single-core Bass/Tile program used SPMD on all cores."""
    if "nc" in _cache:
        return _cache["nc"]

    nc = bacc.Bacc("TRN2", target_bir_lowering=False, debug=False)

    q_d = nc.dram_tensor("q", [HPC * S, D], F32, kind="ExternalInput").ap()
    k_d = nc.dram_tensor("k", [HPC * S, D], F32, kind="ExternalInput").ap()
    v_d = nc.dram_tensor("v", [HPC * S, D], F32, kind="ExternalInput").ap()
    ident_d = nc.dram_tensor("ident", [P, P], F32, kind="ExternalInput").ap()
    tri_d = nc.dram_tensor("tri", [P, P], FP16, kind="ExternalInput").ap()
    o_d = nc.dram_tensor("o", [HPC * S, D], F32, kind="ExternalOutput").ap()

    with tile.TileContext(nc) as tc:
        with (
            tc.tile_pool(name="consts", bufs=1) as consts,
            tc.tile_pool(name="qn", bufs=2) as qn_pool,
            tc.tile_pool(name="kn", bufs=2) as kn_pool,
            tc.tile_pool(name="qt", bufs=2) as qt_pool,
            tc.tile_pool(name="kt", bufs=2) as kt_pool,
            tc.tile_pool(name="vt", bufs=2) as vt_pool,
            tc.tile_pool(name="ptf", bufs=2) as ptf_pool,
            tc.tile_pool(name="ptd", bufs=2) as ptd_pool,
            tc.tile_pool(name="ostage", bufs=2) as ostage_pool,
            tc.tile_pool(name="rec", bufs=4) as rec_pool,
            tc.tile_pool(name="stp", bufs=2, space="PSUM") as st_pool,
            tc.tile_pool(name="ops", bufs=2, space="PSUM") as o_pool,
        ):
            ident = consts.tile([P, P], F32)
            nc.sync.dma_start(ident[:], ident_d[:])
            tri = consts.tile([P, P], FP16)
            nc.sync.dma_start(tri[:], tri_d[:])

            heads = [dict() for _ in range(HPC)]

            def emit_load(h):
                t = heads[h]
                rows = slice(h * S, (h + 1) * S)
                q_h = q_d[rows, :].rearrange("(n p) d -> p n d", p=P)
                k_h = k_d[rows, :].rearrange("(n p) d -> p n d", p=P)
                v_h = v_d[rows, :].rearrange("(n p) d -> p n d", p=P)
                qn = qn_pool.tile([P, NT, P], F32, name=f"qn{h}", tag="qn")
                kn = kn_pool.tile([P, NT, P], F32, name=f"kn{h}", tag="kn")
                vt = vt_pool.tile([P, NT, P + 1], FP16, name=f"vt{h}", tag="vt")
                for c in range(4):
                    cs = slice(4 * c, 4 * c + 4)
                    nc.sync.dma_start(kn[:, cs, :], k_h[:, cs, :])
                    nc.sync.dma_start(qn[:, cs, :], q_h[:, cs, :])
                nc.gpsimd.dma_start(vt[:, :, 0:P], v_h)  # f32 -> fp16 cast DMA
                nc.vector.memset(vt[:, :, P : P + 1], 1.0)
                t["qn"], t["kn"], t["vt"] = qn, kn, vt

            def emit_trans(h):
                t = heads[h]
                qt = qt_pool.tile([P, NT, P], FP16, name=f"qt{h}", tag="qt")
                kt = kt_pool.tile([P, NT, P], FP16, name=f"kt{h}", tag="kt")
                for c in range(4):
                    # [128, 2, 512] f32 = 2 psum banks: q block in bank 0,
                    # k block in bank 1 (copies don't stall transposes)
                    ts = st_pool.tile(
                        [P, 2, QB], F32, name=f"ts{h}_{c}", tag="stp"
                    )
                    for i in range(4):
                        nc.tensor.transpose(
                            ts[:, 0, i * P : (i + 1) * P],
                            t["qn"][:, 4 * c + i, :],
                            ident[:],
                        )
                    for i in range(4):
                        nc.tensor.transpose(
                            ts[:, 1, i * P : (i + 1) * P],
                            t["kn"][:, 4 * c + i, :],
                            ident[:],
                        )
                    cs = slice(4 * c, 4 * c + 4)
                    nc.vector.tensor_copy(qt[:, cs, :], ts[:, 0, :])
                    nc.vector.tensor_copy(kt[:, cs, :], ts[:, 1, :])
                t["qt"], t["kt"] = qt, kt

            def emit_scores(h, b):
                t = heads[h]
                qt, kt = t["qt"], t["kt"]
                qt_b = qt[:, 4 * b : 4 * b + 4, :]  # [128, 512] moving operand
                nfull = 4 * b
                # --- full (entirely valid) k-chunks in G-tile psum groups ---
                ptf = None
                if nfull:
                    ptf = ptf_pool.tile(
                        [P, 12, QB], FP16, name=f"ptf{h}_{b}", tag="ptf"
                    )
                    for gs in range(0, nfull, G):
                        gw = min(G, nfull - gs)
                        stp = st_pool.tile(
                            [P, G, QB], F32, name=f"st{h}_{b}_{gs}", tag="stp"
                        )
                        for jj in range(gw):
                            nc.tensor.matmul(
                                stp[:, jj, :],
                                lhsT=kt[:, gs + jj, :],
                                rhs=qt_b,
                                start=True,
                                stop=True,
                            )
                        nc.scalar.activation(
                            ptf[:, gs : gs + gw, :],
                            stp[:, 0:gw, :],
                            EXPFN,
                            scale=SCALE,
                        )
                # --- diagonal k-chunks: one row-matmul per r, packed tight ---
                # row order (0,3,1,2) packs to exactly 1280 cols with no
                # matmul crossing a 512-f32 psum bank boundary
                dstp = st_pool.tile([P, 1280], F32, name=f"dst{h}_{b}", tag="stp")
                ptd = ptd_pool.tile([P, 1280], FP16, name=f"ptd{h}_{b}", tag="ptd")
                dcol = {}
                col = 0
                for r in (0, 3, 1, 2):
                    w = QB - P * r
                    nc.tensor.matmul(
                        dstp[:, col : col + w],
                        lhsT=kt[:, 4 * b + r, :],
                        rhs=qt_b[:, r:4, :],
                        start=True,
                        stop=True,
                    )
                    dcol[r] = col
                    col += w
                nc.scalar.activation(ptd[:], dstp[:], EXPFN, scale=SCALE)
                for r in range(4):
                    dslc = ptd[:, dcol[r] : dcol[r] + P]
                    nc.vector.tensor_mul(dslc, dslc, tri[:])
                t[("pt", b)] = (ptf, ptd, dcol)

            def emit_out(h, b):
                t = heads[h]
                vt = t["vt"]
                ptf, ptd, dcol = t.pop(("pt", b))
                nfull = 4 * b
                otiles = [
                    o_pool.tile([P, 2, P + 1], F32, name=f"o{h}_{b}_{i}", tag="ops")
                    for i in range(2)
                ]
                for s in range(4):
                    ot = otiles[s // 2][:, s % 2, :]
                    for j in range(nfull):
                        nc.tensor.matmul(
                            ot,
                            lhsT=ptf[:, j, s * P : (s + 1) * P],
                            rhs=vt[:, j, :],
                            start=(j == 0),
                            stop=False,
                        )
                    for r in range(s + 1):
                        c0 = dcol[r] + (s - r) * P
                        nc.tensor.matmul(
                            ot,
                            lhsT=ptd[:, c0 : c0 + P],
                            rhs=vt[:, 4 * b + r, :],
                            start=(nfull == 0 and r == 0),
                            stop=(r == s),
                        )
                rows = slice(h * S, (h + 1) * S)
                o_h = o_d[rows, :].rearrange("(n p) d -> p n d", p=P)
                for s in range(4):
                    g = 4 * b + s
                    ot = otiles[s // 2][:, s % 2, :]
                    rec = rec_pool.tile([P, 1], F32, name=f"r{h}_{g}", tag="rec")
                    nc.vector.reciprocal(rec[:], ot[:, P : P + 1])
                    nc.vector.tensor_scalar_mul(
                        t["ostage"][:, g, :], ot[:, 0:P], rec[:]
                    )
                bs = slice(4 * b, 4 * b + 4)
                nc.sync.dma_start(o_h[:, bs, :], t["ostage"][:, bs, :])

            emit_load(0)
            emit_trans(0)
            pending = None
            for h in range(HPC):
                heads[h]["ostage"] = ostage_pool.tile(
                    [P, NT, P], F32, name=f"os{h}", tag="ostage"
                )
                for b in range(NQB):
                    if b == 3 and h + 1 < HPC:
                        emit_trans(h + 1)
                    emit_scores(h, b)
                    if pending is not None:
                        emit_out(*pending)
                    pending = (h, b)
                    if b == 0 and h + 1 < HPC:
                        emit_load(h + 1)
            emit_out(*pending)

    nc.compile()
    _cache["nc"] = nc
    return nc


def _make_const_inputs():
    ident = np.eye(P, dtype=np.float32)
    # tri[kk, qq] = 1 where qq >= kk (valid causal positions in S^T layout)
    tri = np.triu(np.ones((P, P), dtype=np.float16))
    return ident, tri


def run_sharded(q, k, v, trace=False, **kw):
    """q,k,v: [B,H,S,D] f32 -> (out [B,H,S,D] f32, BassKernelResults)."""
    nc = _build_program()
    qf = np.ascontiguousarray(np.asarray(q, dtype=np.float32).reshape(B * H, S, D))
    kf = np.ascontiguousarray(np.asarray(k, dtype=np.float32).reshape(B * H, S, D))
    vf = np.ascontiguousarray(np.asarray(v, dtype=np.float32).reshape(B * H, S, D))
    ident, tri = _make_const_inputs()
    in_maps = []
    for c in range(N_CORES):
        hs = slice(c * HPC, (c + 1) * HPC)
        in_maps.append(
            {
                "q": qf[hs].reshape(HPC * S, D),
                "k": kf[hs].reshape(HPC * S, D),
                "v": vf[hs].reshape(HPC * S, D),
                "ident": ident,
                "tri": tri,
            }
        )
    res = run_bass_kernel_spmd(nc, in_maps, list(range(N_CORES)), trace=trace, **kw)
    outs = [res.results[c]["o"].reshape(HPC, S, D) for c in range(N_CORES)]
    full = np.concatenate(outs, axis=0).reshape(B, H, S, D)
    return full, res


def kernel(query_states, key_states, value_states):
    out, _ = run_sharded(query_states, key_states, value_states)
    return out.astype(np.float32)


# revision 10
# speedup vs baseline: 1.0674x; 1.0201x over previous
"""Causal multi-head attention (B=2, H=16, S=2048, D=128) on 8 TRN2 NeuronCores.

Sharding: batch*heads (32) split across 8 cores, 4 heads per core.
Per-head algorithm (fp16 matmuls / f32 accumulation), v3:
  - chunked HWDGE f32 loads for Q,K (descriptor gen off the GpSimd Q7);
    V loaded via one SWDGE cast-DMA (f32 -> fp16) with a ones column appended
  - PE-transpose Q,K 128x128 f32 tiles to [d, s] layout; the PSUM->SBUF copy
    casts to fp16 (DVE)
  - scores computed transposed: S^T[k, q] so the PV matmul needs no P transpose
  - full k-chunks in 3-tile PSUM groups [128,1536]; the 4 diagonal k-chunks of
    each 512-q block as 4 row-matmuls packed tight [128,1280] (valid cols only)
  - P^T = exp(S^T/sqrt(D)) on ScalarE straight from PSUM -> SBUF fp16; no
    max-subtraction needed (scores ~N(0,1); the reference's -10000 mask
    underflows to exact 0 in exp, so hard zeros match it)
  - causal diagonal tiles masked by a 0/1 triangle multiply (DVE) after exp
  - O = sum_k P^T.T @ V_aug with a ones column appended to V -> last column of
    the accumulator is the softmax denominator; DVE reciprocal + tensor_scalar
  - per-q-block f32 stores; O-phase of each block emitted one scores-unit late
    so PE keeps ScalarE fed across block/head boundaries
"""

import math

import numpy as np

import concourse.bass as bass
import concourse.tile as tile
from concourse import bacc, mybir
from concourse.bass_utils import run_bass_kernel_spmd

B, H, S, D = 2, 16, 2048, 128
N_CORES = 8
HPC = (B * H) // N_CORES  # heads per core
P = 128                   # partitions / head_dim / k-chunk
NT = S // P               # 16 k-chunks (s-tiles) per head
QB = 512                  # q-block width
NQB = S // QB             # 4 q-blocks per head
G = 3                     # full-group tiles per exp (3 PSUM banks)

FP16 = mybir.dt.float16
BF16 = mybir.dt.bfloat16
F32 = mybir.dt.float32
EXPFN = mybir.ActivationFunctionType.Exp
SCALE = 1.0 / math.sqrt(D)

_cache = {}


def _build_program():
    """Build (once) the single-core Bass/Tile program used SPMD on all cores."""
    if "nc" in _cache:
        return _cache["nc"]

    nc = bacc.Bacc("TRN2", target_bir_lowering=False, debug=False)

    q_d = nc.dram_tensor("q", [HPC * S, D], F32, kind="ExternalInput").ap()
    k_d = nc.dram_tensor("k", [HPC * S, D], F32, kind="ExternalInput").ap()
    v_d = nc.dram_tensor("v", [HPC * S, D], F32, kind="ExternalInput").ap()
    ident_d = nc.dram_tensor("ident", [P, P], F32, kind="ExternalInput").ap()
    tri_d = nc.dram_tensor("tri", [P, P], BF16, kind="ExternalInput").ap()
    o_d = nc.dram_tensor("o", [HPC * S, D], F32, kind="ExternalOutput").ap()

    with tile.TileContext(nc) as tc:
        with (
            tc.tile_pool(name="consts", bufs=1) as consts,
            tc.tile_pool(name="qn", bufs=6) as qn_pool,
            tc.tile_pool(name="kn", bufs=6) as kn_pool,
            tc.tile_pool(name="qt", bufs=8) as qt_pool,
            tc.tile_pool(name="kt", bufs=8) as kt_pool,
            tc.tile_pool(name="vt", bufs=8) as vt_pool,
            tc.tile_pool(name="ptf", bufs=2) as ptf_pool,
            tc.tile_pool(name="ptd", bufs=2) as ptd_pool,
            tc.tile_pool(name="ostage", bufs=4) as ostage_pool,
            tc.tile_pool(name="rec", bufs=4) as rec_pool,
            tc.tile_pool(name="stp", bufs=2, space="PSUM") as st_pool,
            tc.tile_pool(name="ops", bufs=2, space="PSUM") as o_pool,
        ):
            ident = consts.tile([P, P], F32)
            nc.sync.dma_start(ident[:], ident_d[:])
            tri = consts.tile([P, P], BF16)
            nc.sync.dma_start(tri[:], tri_d[:])

            heads = [dict() for _ in range(HPC)]

            def emit_load(h):
                t = heads[h]
                rows = slice(h * S, (h + 1) * S)
                q_h = q_d[rows, :].rearrange("(n p) d -> p n d", p=P)
                k_h = k_d[rows, :].rearrange("(n p) d -> p n d", p=P)
                v_h = v_d[rows, :].rearrange("(n p) d -> p n d", p=P)
                qn, kn, vt = [], [], []
                for c in range(4):
                    cs = slice(4 * c, 4 * c + 4)
                    knc = kn_pool.tile([P, 4, P], F32, name=f"kn{h}_{c}", tag="kn")
                    nc.sync.dma_start(knc[:], k_h[:, cs, :])
                    qnc = qn_pool.tile([P, 4, P], F32, name=f"qn{h}_{c}", tag="qn")
                    nc.sync.dma_start(qnc[:], q_h[:, cs, :])
                    vtc = vt_pool.tile([P, 4, P + 1], BF16, name=f"vt{h}_{c}", tag="vt")
                    nc.gpsimd.dma_start(vtc[:, :, 0:P], v_h[:, cs, :])  # f32->bf16
                    nc.vector.memset(vtc[:, :, P : P + 1], 1.0)
                    qn.append(qnc)
                    kn.append(knc)
                    vt.append(vtc)
                t["qn"], t["kn"], t["vt"] = qn, kn, vt

            def emit_trans_chunk(h, c):
                t = heads[h]
                if c == 0:
                    t["qt"], t["kt"] = [], []
                # [128, 2, 512] f32 = 2 psum banks: q block in bank 0,
                # k block in bank 1 (copies don't stall transposes)
                ts = st_pool.tile([P, 2, QB], F32, name=f"ts{h}_{c}", tag="stp")
                for i in range(4):
                    nc.tensor.transpose(
                        ts[:, 0, i * P : (i + 1) * P],
                        t["qn"][c][:, i, :],
                        ident[:],
                    )
                for i in range(4):
                    nc.tensor.transpose(
                        ts[:, 1, i * P : (i + 1) * P],
                        t["kn"][c][:, i, :],
                        ident[:],
                    )
                qtc = qt_pool.tile([P, 4, P], BF16, name=f"qt{h}_{c}", tag="qt")
                ktc = kt_pool.tile([P, 4, P], BF16, name=f"kt{h}_{c}", tag="kt")
                nc.vector.tensor_copy(qtc[:], ts[:, 0, :])
                nc.vector.tensor_copy(ktc[:], ts[:, 1, :])
                t["qt"].append(qtc)
                t["kt"].append(ktc)

            def emit_scores(h, b):
                t = heads[h]
                qt, kt = t["qt"], t["kt"]
                qt_b = qt[b]  # [128, 4, 128] = [128, 512] moving operand
                nfull = 4 * b
                # --- full (entirely valid) k-chunks in G-tile psum groups ---
                ptf = None
                if nfull:
                    ptf = ptf_pool.tile(
                        [P, 12, QB], BF16, name=f"ptf{h}_{b}", tag="ptf"
                    )
                    for gs in range(0, nfull, G):
                        gw = min(G, nfull - gs)
                        stp = st_pool.tile(
                            [P, G, QB], F32, name=f"st{h}_{b}_{gs}", tag="stp"
                        )
                        for jj in range(gw):
                            nc.tensor.matmul(
                                stp[:, jj, :],
                                lhsT=kt[(gs + jj) // 4][:, (gs + jj) % 4, :],
                                rhs=qt_b[:],
                                start=True,
                                stop=True,
                            )
                        nc.scalar.activation(
                            ptf[:, gs : gs + gw, :],
                            stp[:, 0:gw, :],
                            EXPFN,
                            scale=SCALE,
                        )
                # --- diagonal k-chunks: one row-matmul per r, packed tight ---
                # row order (0,3,1,2) packs to exactly 1280 cols with no
                # matmul crossing a 512-f32 psum bank boundary
                dstp = st_pool.tile([P, 1280], F32, name=f"dst{h}_{b}", tag="stp")
                ptd = ptd_pool.tile([P, 1280], BF16, name=f"ptd{h}_{b}", tag="ptd")
                dcol = {}
                col = 0
                for r in (0, 3, 1, 2):
                    w = QB - P * r
                    nc.tensor.matmul(
                        dstp[:, col : col + w],
                        lhsT=kt[b][:, r, :],
                        rhs=qt_b[:, r:4, :],
                        start=True,
                        stop=True,
                    )
                    dcol[r] = col
                    col += w
                nc.scalar.activation(ptd[:], dstp[:], EXPFN, scale=SCALE)
                for r in range(4):
                    dslc = ptd[:, dcol[r] : dcol[r] + P]
                    nc.gpsimd.tensor_mul(dslc, dslc, tri[:])
                t[("pt", b)] = (ptf, ptd, dcol)

            def emit_out(h, b):
                t = heads[h]
                vt = t["vt"]
                ptf, ptd, dcol = t.pop(("pt", b))
                ostage = ostage_pool.tile(
                    [P, 4, P], F32, name=f"os{h}_{b}", tag="ostage"
                )
                nfull = 4 * b
                otiles = [
                    o_pool.tile([P, 2, P + 1], F32, name=f"o{h}_{b}_{i}", tag="ops")
                    for i in range(2)
                ]
                for s in range(4):
                    ot = otiles[s // 2][:, s % 2, :]
                    for j in range(nfull):
                        nc.tensor.matmul(
                            ot,
                            lhsT=ptf[:, j, s * P : (s + 1) * P],
                            rhs=vt[j // 4][:, j % 4, :],
                            start=(j == 0),
                            stop=False,
                        )
                    for r in range(s + 1):
                        c0 = dcol[r] + (s - r) * P
                        nc.tensor.matmul(
                            ot,
                            lhsT=ptd[:, c0 : c0 + P],
                            rhs=vt[b][:, r, :],
                            start=(nfull == 0 and r == 0),
                            stop=(r == s),
                        )
                rows = slice(h * S, (h + 1) * S)
                o_h = o_d[rows, :].rearrange("(n p) d -> p n d", p=P)
                for s in range(4):
                    ot = otiles[s // 2][:, s % 2, :]
                    rec = rec_pool.tile([P, 1], F32, name=f"r{h}_{b}_{s}", tag="rec")
                    nc.vector.reciprocal(rec[:], ot[:, P : P + 1])
                    nc.vector.tensor_scalar_mul(
                        ostage[:, s, :], ot[:, 0:P], rec[:]
                    )
                bs = slice(4 * b, 4 * b + 4)
                nc.sync.dma_start(o_h[:, bs, :], ostage[:])

            emit_load(0)
            for c in range(4):
                emit_trans_chunk(0, c)
            pending = None
            # transpose chunks of head h+1 spread over blocks 1..3 of head h
            tsched = {1: [0], 2: [1, 2], 3: [3]}
            for h in range(HPC):
                for b in range(NQB):
                    if h + 1 < HPC:
                        for c in tsched.get(b, []):
                            emit_trans_chunk(h + 1, c)
                    emit_scores(h, b)
                    if pending is not None:
                        emit_out(*pending)
                    pending = (h, b)
                    if b == 0 and h + 1 < HPC:
                        emit_load(h + 1)
            emit_out(*pending)

    nc.compile()
    _cache["nc"] = nc
    return nc


def _make_const_inputs():
    import ml_dtypes

    ident = np.eye(P, dtype=np.float32)
    # tri[kk, qq] = 1 where qq >= kk (valid causal positions in S^T layout)
    tri = np.triu(np.ones((P, P), dtype=ml_dtypes.bfloat16))
    return ident, tri


def run_sharded(q, k, v, trace=False, **kw):
    """q,k,v: [B,H,S,D] f32 -> (out [B,H,S,D] f32, BassKernelResults)."""
    nc = _build_program()
    qf = np.ascontiguousarray(np.asarray(q, dtype=np.float32).reshape(B * H, S, D))
    kf = np.ascontiguousarray(np.asarray(k, dtype=np.float32).reshape(B * H, S, D))
    vf = np.ascontiguousarray(np.asarray(v, dtype=np.float32).reshape(B * H, S, D))
    ident, tri = _make_const_inputs()
    in_maps = []
    for c in range(N_CORES):
        hs = slice(c * HPC, (c + 1) * HPC)
        in_maps.append(
            {
                "q": qf[hs].reshape(HPC * S, D),
                "k": kf[hs].reshape(HPC * S, D),
                "v": vf[hs].reshape(HPC * S, D),
                "ident": ident,
                "tri": tri,
            }
        )
    res = run_bass_kernel_spmd(nc, in_maps, list(range(N_CORES)), trace=trace, **kw)
    outs = [res.results[c]["o"].reshape(HPC, S, D) for c in range(N_CORES)]
    full = np.concatenate(outs, axis=0).reshape(B, H, S, D)
    return full, res


def kernel(query_states, key_states, value_states):
    out, _ = run_sharded(query_states, key_states, value_states)
    return out.astype(np.float32)
